# revision 1
# baseline (speedup 1.0000x reference)
"""DiceLoss partial-sum kernel for Trainium2 (8 NeuronCores, data-parallel).

Computes, for input/target of shape (32, 1, 1024, 1024) fp32:
    bin   = (input > 0.5) ? 1.0 : 0.0
    loss1 = 2 * sum(bin * target)
    loss2 = sum(bin) + sum(target)
and returns (loss1, loss2) as fp32 scalars (same structure as the reference).

Sharding: batch dim N=32 is split 4-per-core across 8 cores. Each core
streams its 16 MiB input + 16 MiB target shard through SBUF as [128, F]
fp32 tiles via HWDGE DMA. The problem is HBM-bound (~81 us of DMA per core
at the ~410 GB/s fair share), so compute is split across engines to stay
off the critical path, and the kernel is written in raw bacc (hand-rolled
semaphores, no TileContext) with a flat 3-stage pipeline:
  sync:   paced tile loads into a BUFS-slot SBUF ring (per-slot semaphores —
          HWDGE completions are NOT ordered across dma_starts)
  vector: every tile: STT (in>0.5)*tgt, accum -> loss1 column;
          "dve" tiles also STT (in>0.5)+tgt, accum -> loss2 column (exact)
  scalar: "act" tiles: Copy(tgt) accum -> tgt column and
          Sign(1-2*in) accum -> sign column (bin recovered on host as
          (count - S')/2; exact up to elements equal to 0.5, whose
          contribution error is ~1e-7 relative)
  gpsimd: zeroes the stats tile once at start
The dve/act split balances VectorE vs ScalarE, both well under DMA; the
last two tiles are 1024 wide so the final load's compute tail is ~2-3 us. Per-partition partial sums land in a [128, 3*nt] stats
tile DMA'd out per core; the final tiny reduction over cores/partitions/
tiles happens on the host in float64.

Measured on trn2 (NTFF, core 0, steady state): 96.8-97.7 us, highly
stable across runs. Budget: ~12.5-14 us fixed NEFF entry/exit (measured
with an empty kernel) + ~80.5 us HBM-saturated bulk + ~2.5 us ramp/tail.
"""

from contextlib import ExitStack

import numpy as np

try:
    import concourse.bass  # noqa: F401
except ImportError:  # pragma: no cover - path fallback for bare containers
    import sys

    for _p in ("/opt/trn_rl_repo", "/root/.axon_site/_ro/trn_rl_repo"):
        if _p not in sys.path:
            sys.path.insert(0, _p)

import concourse.bacc as bacc
import concourse.mybir as mybir
from concourse.bass_utils import run_bass_kernel_spmd

N_CORES = 8
FULL_SHAPE = (32, 1, 1024, 1024)
FULL_ELEMS = 32 * 1024 * 1024
PER_CORE = FULL_ELEMS // N_CORES  # 4_194_304
P = 128
FREE = PER_CORE // P  # 32768 fp32 elements per partition per tensor
THRESH = 0.5
BUFS = 4  # SBUF ring depth per tensor (4 x 16 KiB rows per partition)

# (free_size, loss2_mode) per tile; sizes must sum to FREE. 4096-wide tiles
# (2 MiB DMAs, 16 KB descriptor rows) proved much more robust to cross-core
# HBM contention than 2048-wide ones (fewer dma_starts -> longer descriptor
# runway per issue; the 2048 variant intermittently stretched its bulk phase
# ~18% from engine starvation). The tail tapers so the final load's compute
# is short.
TILES = tuple(
    [(4096, "act"), (4096, "act"), (4096, "dve"), (4096, "act"),
     (4096, "act"), (4096, "dve"), (4096, "act"),
     (2048, "act"), (1024, "act"), (1024, "dve")]
)
assert sum(f for f, _ in TILES) == FREE

_CACHE: dict = {}


def _build(tiles: tuple, n_cores: int):
    f32 = mybir.dt.float32
    nt = len(tiles)
    per_core = P * sum(f for f, _ in tiles)
    max_f = max(f for f, _ in tiles)
    nc = bacc.Bacc(
        "TRN2", target_bir_lowering=False, debug=False, num_devices=n_cores
    )
    inp = nc.dram_tensor("input", [per_core], f32, kind="ExternalInput").ap()
    tgt = nc.dram_tensor("target", [per_core], f32, kind="ExternalInput").ap()
    stats = nc.dram_tensor("stats", [P, 3 * nt], f32, kind="ExternalOutput").ap()

    ti_ring = nc.alloc_sbuf_tensor("ti_ring", [P, BUFS * max_f], f32).ap()
    tt_ring = nc.alloc_sbuf_tensor("tt_ring", [P, BUFS * max_f], f32).ap()
    # two scratch outputs per engine, alternated so consecutive same-engine
    # instructions never write the same buffer (deep-pipeline WAW)
    sd = [nc.alloc_sbuf_tensor(f"sd{i}", [P, max_f], f32).ap() for i in range(2)]
    sa = [nc.alloc_sbuf_tensor(f"sa{i}", [P, max_f], f32).ap() for i in range(2)]
    st = nc.alloc_sbuf_tensor("st", [P, 3 * nt], f32).ap()

    # per-tile DRAM offsets and cumulative consumer-instruction counts
    offs = []
    off = 0
    for tf, _ in tiles:
        offs.append(off)
        off += P * tf
    V = []  # vector instr count through tile t
    S = []  # scalar instr count through tile t
    v = s = 0
    for tf, mode in tiles:
        v += 2 if mode == "dve" else 1
        s += 0 if mode == "dve" else 2
        V.append(v)
        S.append(s)

    with ExitStack() as ctx:
        slot_sems = [
            ctx.enter_context(nc.semaphore(f"slot_sem{i}")) for i in range(BUFS)
        ]
        vec_sem = ctx.enter_context(nc.semaphore("vec_sem"))
        sc_sem = ctx.enter_context(nc.semaphore("sc_sem"))
        gp_sem = ctx.enter_context(nc.semaphore("gp_sem"))
        out_sem = ctx.enter_context(nc.semaphore("out_sem"))
        block = ctx.enter_context(nc.Block())

        @block.gpsimd
        def _(gpsimd):
            gpsimd.memset(st[:], 0.0).then_inc(gp_sem, 1)

        @block.sync
        def _(sync):
            for t, (tf, mode) in enumerate(tiles):
                s_ = (t % BUFS) * max_f
                if t >= BUFS:
                    # ring slot reuse: consumers of tile t-BUFS must be done
                    sync.wait_ge(vec_sem, V[t - BUFS])
                    if S[t - BUFS] > 0:
                        sync.wait_ge(sc_sem, S[t - BUFS])
                src_i = inp[offs[t] : offs[t] + P * tf].rearrange(
                    "(p f) -> p f", p=P
                )
                src_t = tgt[offs[t] : offs[t] + P * tf].rearrange(
                    "(p f) -> p f", p=P
                )
                sem = slot_sems[t % BUFS]
                sync.dma_start(out=ti_ring[:, s_ : s_ + tf], in_=src_i).then_inc(
                    sem, 16
                )
                sync.dma_start(out=tt_ring[:, s_ : s_ + tf], in_=src_t).then_inc(
                    sem, 16
                )
            # sem update on an accum instruction fires at full instruction
            # retirement (incl. the accumulator write-back), so the stats DMA
            # can depend on the compute sems directly - no fence instructions
            sync.wait_ge(vec_sem, V[-1])
            sync.wait_ge(sc_sem, S[-1])
            sync.wait_ge(gp_sem, 1)
            sync.dma_start(out=stats[:], in_=st[:]).then_inc(out_sem, 16)
            sync.wait_ge(out_sem, 16)

        @block.vector
        def _(vector):
            vector.wait_ge(gp_sem, 1)
            vi = 0
            for t, (tf, mode) in enumerate(tiles):
                s_ = (t % BUFS) * max_f
                vector.wait_ge(slot_sems[t % BUFS], 32 * (t // BUFS + 1))
                if vi >= 2:
                    # scratch-reuse self-wait; satisfied by in-order retirement
                    vector.wait_ge(vec_sem, vi - 1)
                vector.scalar_tensor_tensor(
                    out=sd[vi % 2][:, :tf],
                    in0=ti_ring[:, s_ : s_ + tf],
                    scalar=THRESH,
                    in1=tt_ring[:, s_ : s_ + tf],
                    op0=mybir.AluOpType.is_gt,
                    op1=mybir.AluOpType.mult,
                    accum_out=st[:, t : t + 1],
                ).then_inc(vec_sem, 1)
                vi += 1
                if mode == "dve":
                    if vi >= 2:
                        vector.wait_ge(vec_sem, vi - 1)
                    vector.scalar_tensor_tensor(
                        out=sd[vi % 2][:, :tf],
                        in0=ti_ring[:, s_ : s_ + tf],
                        scalar=THRESH,
                        in1=tt_ring[:, s_ : s_ + tf],
                        op0=mybir.AluOpType.is_gt,
                        op1=mybir.AluOpType.add,
                        accum_out=st[:, nt + t : nt + t + 1],
                    ).then_inc(vec_sem, 1)
                    vi += 1


        @block.scalar
        def _(scalar):
            scalar.wait_ge(gp_sem, 1)
            si = 0
            for t, (tf, mode) in enumerate(tiles):
                if mode == "dve":
                    continue
                s_ = (t % BUFS) * max_f
                scalar.wait_ge(slot_sems[t % BUFS], 32 * (t // BUFS + 1))
                if si >= 2:
                    scalar.wait_ge(sc_sem, si - 1)
                scalar.activation(
                    out=sa[0][:, :tf],
                    in_=tt_ring[:, s_ : s_ + tf],
                    func=mybir.ActivationFunctionType.Copy,
                    accum_out=st[:, nt + t : nt + t + 1],
                ).then_inc(sc_sem, 1)
                si += 1
                if si >= 2:
                    scalar.wait_ge(sc_sem, si - 1)
                # Sign(1 - 2x) = -Sign(x - 0.5); bias=1.0 has a pre-registered
                # const AP, the host negates
                scalar.activation(
                    out=sa[1][:, :tf],
                    in_=ti_ring[:, s_ : s_ + tf],
                    func=mybir.ActivationFunctionType.Sign,
                    bias=1.0,
                    scale=-2.0,
                    accum_out=st[:, 2 * nt + t : 2 * nt + t + 1],
                ).then_inc(sc_sem, 1)
                si += 1


    nc.compile()
    return nc


def _get_nc():
    key = (TILES, N_CORES)
    if key not in _CACHE:
        _CACHE[key] = _build(*key)
    return _CACHE[key]


def kernel(input: np.ndarray, target: np.ndarray, **run_kwargs):
    inp = np.asarray(input, dtype=np.float32).reshape(N_CORES, PER_CORE)
    tgt = np.asarray(target, dtype=np.float32).reshape(N_CORES, PER_CORE)

    nc = _get_nc()
    in_maps = [
        {"input": np.ascontiguousarray(inp[c]), "target": np.ascontiguousarray(tgt[c])}
        for c in range(N_CORES)
    ]
    res = run_bass_kernel_spmd(nc, in_maps, core_ids=list(range(N_CORES)), **run_kwargs)

    nt = len(TILES)
    act_tiles = [t for t, (_, m) in enumerate(TILES) if m == "act"]
    inter = 0.0
    loss2 = 0.0
    sign_sum = 0.0
    for c in range(N_CORES):
        stats = res.results[c]["stats"].astype(np.float64)
        inter += stats[:, :nt].sum()
        # "dve" tiles: direct (bin + tgt) partials; "act" tiles: Copy -> tgt sums
        loss2 += stats[:, nt : 2 * nt].sum()
        sign_sum += sum(stats[:, 2 * nt + t].sum() for t in act_tiles)
    # "act" tiles' bin count from sign sums: S' = #lt - #gt -> bin = (n - S')/2
    n_act_elems = N_CORES * P * sum(TILES[t][0] for t in act_tiles)
    loss2 += (n_act_elems - sign_sum) / 2.0

    loss1 = np.float32(2.0 * inter)
    loss2 = np.float32(loss2)
    out = (loss1, loss2)
    if run_kwargs.get("trace"):
        return out, res
    return out



# revision 2
# speedup vs baseline: 1.1244x; 1.1244x over previous
"""DiceLoss partial-sum kernel for Trainium2 (8 NeuronCores, data-parallel).

Computes, for input/target of shape (32, 1, 1024, 1024) fp32:
    bin   = (input > 0.5) ? 1.0 : 0.0
    loss1 = 2 * sum(bin * target)
    loss2 = sum(bin) + sum(target)
and returns (loss1, loss2) as fp32 scalars (same structure as the reference).

Sharding: batch dim N=32 is split 4-per-core across 8 cores. Each core
streams its 16 MiB input + 16 MiB target shard through SBUF as [128, F]
fp32 tiles via HWDGE DMA. The problem is HBM-bound: the 16 HW DMA engines
sustain ~425-430 GB/s per core (the profiler's dma_ddr_bandwidth cap is
435 GB/s), so the whole design keeps every engine's per-tile work well
under the ~9.5 us/tile DMA pace so compute never trails the data:
  sync:   paced tile loads into a BUFS-slot SBUF ring (per-slot semaphores -
          HWDGE completions are NOT ordered across dma_starts)
  vector: per tile: STT (in>0.5)*tgt, accum -> loss1 column (4.4 us/4096)
          + 1-input tensor_reduce sum(tgt) -> tgt column (2.1 us/4096)
  scalar: per tile: Sign(1-2*in) accum -> sign column (3.7 us/4096); bin
          count recovered on host as (count - S')/2, exact up to elements
          equal to 0.5 (~1e-8 relative)
  gpsimd: zeroes the stats tile once at start
The previous engine split (scalar Copy+Sign on 7 tiles, double-STT on 3)
left a ~20 us vector/scalar tail after the last DMA byte; this split's
per-tile maximum is vector's 6.6 us < 9.5 us pace, so the tail is just the
final (tapered, 1024-wide) tile's ~2 us. Per-partition partial sums land
in a [128, 3*nt] stats tile DMA'd out per core; the final tiny reduction
over cores/partitions/tiles happens on the host in float64.
"""

from contextlib import ExitStack

import numpy as np

try:
    import concourse.bass  # noqa: F401
except ImportError:  # pragma: no cover - path fallback for bare containers
    import sys

    for _p in ("/opt/trn_rl_repo", "/root/.axon_site/_ro/trn_rl_repo"):
        if _p not in sys.path:
            sys.path.insert(0, _p)

import concourse.bacc as bacc
import concourse.mybir as mybir
from concourse.bass_utils import run_bass_kernel_spmd

N_CORES = 8
FULL_SHAPE = (32, 1, 1024, 1024)
FULL_ELEMS = 32 * 1024 * 1024
PER_CORE = FULL_ELEMS // N_CORES  # 4_194_304
P = 128
FREE = PER_CORE // P  # 32768 fp32 elements per partition per tensor
THRESH = 0.5
BUFS = 4  # SBUF ring depth per tensor (4 x 16 KiB rows per partition)

# free_size per tile; sizes must sum to FREE. 4096-wide tiles (2 MiB DMAs,
# 16 KB descriptor rows) proved much more robust to cross-core HBM
# contention than 2048-wide ones. The tail tapers so the final load's
# compute is short.
TILES = (4096, 4096, 4096, 4096, 4096, 4096, 4096, 2048, 1024, 1024)
assert sum(TILES) == FREE

_CACHE: dict = {}


def _build(tiles: tuple, n_cores: int):
    f32 = mybir.dt.float32
    nt = len(tiles)
    per_core = P * sum(tiles)
    max_f = max(tiles)
    nc = bacc.Bacc(
        "TRN2", target_bir_lowering=False, debug=False, num_devices=n_cores
    )
    inp = nc.dram_tensor("input", [per_core], f32, kind="ExternalInput").ap()
    tgt = nc.dram_tensor("target", [per_core], f32, kind="ExternalInput").ap()
    stats = nc.dram_tensor("stats", [P, 3 * nt], f32, kind="ExternalOutput").ap()

    ti_ring = nc.alloc_sbuf_tensor("ti_ring", [P, BUFS * max_f], f32).ap()
    tt_ring = nc.alloc_sbuf_tensor("tt_ring", [P, BUFS * max_f], f32).ap()
    # two scratch outputs per engine, alternated so consecutive same-engine
    # instructions never write the same buffer (deep-pipeline WAW)
    sd = [nc.alloc_sbuf_tensor(f"sd{i}", [P, max_f], f32).ap() for i in range(2)]
    sa = [nc.alloc_sbuf_tensor(f"sa{i}", [P, max_f], f32).ap() for i in range(2)]
    st = nc.alloc_sbuf_tensor("st", [P, 3 * nt], f32).ap()

    # per-tile DRAM offsets and cumulative consumer-instruction counts
    offs = []
    off = 0
    for tf in tiles:
        offs.append(off)
        off += P * tf
    V = [2 * (t + 1) for t in range(nt)]  # vector instrs through tile t
    S = [t + 1 for t in range(nt)]  # scalar instrs through tile t

    with ExitStack() as ctx:
        slot_sems = [
            ctx.enter_context(nc.semaphore(f"slot_sem{i}")) for i in range(BUFS)
        ]
        vec_sem = ctx.enter_context(nc.semaphore("vec_sem"))
        sc_sem = ctx.enter_context(nc.semaphore("sc_sem"))
        gp_sem = ctx.enter_context(nc.semaphore("gp_sem"))
        out_sem = ctx.enter_context(nc.semaphore("out_sem"))
        block = ctx.enter_context(nc.Block())

        @block.gpsimd
        def _(gpsimd):
            gpsimd.memset(st[:], 0.0).then_inc(gp_sem, 1)

        @block.sync
        def _(sync):
            for t, tf in enumerate(tiles):
                s_ = (t % BUFS) * max_f
                if t >= BUFS:
                    # ring slot reuse: consumers of tile t-BUFS must be done
                    sync.wait_ge(vec_sem, V[t - BUFS])
                    sync.wait_ge(sc_sem, S[t - BUFS])
                src_i = inp[offs[t] : offs[t] + P * tf].rearrange(
                    "(p f) -> p f", p=P
                )
                src_t = tgt[offs[t] : offs[t] + P * tf].rearrange(
                    "(p f) -> p f", p=P
                )
                sem = slot_sems[t % BUFS]
                sync.dma_start(out=ti_ring[:, s_ : s_ + tf], in_=src_i).then_inc(
                    sem, 16
                )
                sync.dma_start(out=tt_ring[:, s_ : s_ + tf], in_=src_t).then_inc(
                    sem, 16
                )
            # sem update on an accum instruction fires at full instruction
            # retirement (incl. the accumulator write-back), so the stats DMA
            # can depend on the compute sems directly - no fence instructions
            sync.wait_ge(vec_sem, V[-1])
            sync.wait_ge(sc_sem, S[-1])
            sync.wait_ge(gp_sem, 1)
            sync.dma_start(out=stats[:], in_=st[:]).then_inc(out_sem, 16)
            sync.wait_ge(out_sem, 16)

        @block.vector
        def _(vector):
            vector.wait_ge(gp_sem, 1)
            vi = 0
            for t, tf in enumerate(tiles):
                s_ = (t % BUFS) * max_f
                vector.wait_ge(slot_sems[t % BUFS], 32 * (t // BUFS + 1))
                if vi >= 4:
                    # scratch-reuse self-wait: the STT two tiles back (4
                    # vector instrs ago) must have retired before its sd
                    # buffer is rewritten
                    vector.wait_ge(vec_sem, vi - 3)
                vector.scalar_tensor_tensor(
                    out=sd[t % 2][:, :tf],
                    in0=ti_ring[:, s_ : s_ + tf],
                    scalar=THRESH,
                    in1=tt_ring[:, s_ : s_ + tf],
                    op0=mybir.AluOpType.is_gt,
                    op1=mybir.AluOpType.mult,
                    accum_out=st[:, t : t + 1],
                ).then_inc(vec_sem, 1)
                vi += 1
                # 1-input free-axis sum of the raw target tile -> tgt column
                # (writes its own st column; no scratch, no WAW)
                vector.tensor_reduce(
                    out=st[:, 2 * nt + t : 2 * nt + t + 1],
                    in_=tt_ring[:, s_ : s_ + tf],
                    axis=mybir.AxisListType.X,
                    op=mybir.AluOpType.add,
                ).then_inc(vec_sem, 1)
                vi += 1

        @block.scalar
        def _(scalar):
            scalar.wait_ge(gp_sem, 1)
            si = 0
            for t, tf in enumerate(tiles):
                s_ = (t % BUFS) * max_f
                scalar.wait_ge(slot_sems[t % BUFS], 32 * (t // BUFS + 1))
                if si >= 2:
                    scalar.wait_ge(sc_sem, si - 1)
                # Sign(1 - 2x) = -Sign(x - 0.5); bias=1.0 has a pre-registered
                # const AP; host converts the sum to a >0.5 count
                scalar.activation(
                    out=sa[si % 2][:, :tf],
                    in_=ti_ring[:, s_ : s_ + tf],
                    func=mybir.ActivationFunctionType.Sign,
                    bias=1.0,
                    scale=-2.0,
                    accum_out=st[:, nt + t : nt + t + 1],
                ).then_inc(sc_sem, 1)
                si += 1

    nc.compile()
    return nc


def _get_nc():
    key = (TILES, N_CORES)
    if key not in _CACHE:
        _CACHE[key] = _build(*key)
    return _CACHE[key]


def kernel(input: np.ndarray, target: np.ndarray, **run_kwargs):
    inp = np.asarray(input, dtype=np.float32).reshape(N_CORES, PER_CORE)
    tgt = np.asarray(target, dtype=np.float32).reshape(N_CORES, PER_CORE)

    nc = _get_nc()
    in_maps = [
        {"input": np.ascontiguousarray(inp[c]), "target": np.ascontiguousarray(tgt[c])}
        for c in range(N_CORES)
    ]
    res = run_bass_kernel_spmd(nc, in_maps, core_ids=list(range(N_CORES)), **run_kwargs)

    nt = len(TILES)
    inter = 0.0
    sign_sum = 0.0
    tgt_sum = 0.0
    for c in range(N_CORES):
        stats = res.results[c]["stats"].astype(np.float64)
        inter += stats[:, :nt].sum()
        sign_sum += stats[:, nt : 2 * nt].sum()
        tgt_sum += stats[:, 2 * nt :].sum()
    # bin count from sign sums: S' = #lt - #gt -> count(>thr) = (n - S')/2
    loss2 = tgt_sum + (FULL_ELEMS - sign_sum) / 2.0

    loss1 = np.float32(2.0 * inter)
    loss2 = np.float32(loss2)
    out = (loss1, loss2)
    if run_kwargs.get("trace"):
        return out, res
    return out


# revision 3
# speedup vs baseline: 1.1267x; 1.0020x over previous
"""DiceLoss partial-sum kernel for Trainium2 (8 NeuronCores, data-parallel).

Computes, for input/target of shape (32, 1, 1024, 1024) fp32:
    bin   = (input > 0.5) ? 1.0 : 0.0
    loss1 = 2 * sum(bin * target)
    loss2 = sum(bin) + sum(target)
and returns (loss1, loss2) as fp32 scalars (same structure as the reference).

Sharding: batch dim N=32 is split 4-per-core across 8 cores. Each core
streams its 16 MiB input + 16 MiB target shard through SBUF as [128, F]
fp32 tiles via HWDGE DMA. The problem is HBM-bound: the 16 HW DMA engines
sustain ~429 GB/s per core (profiler dma_ddr_bandwidth cap: 435), i.e. a
~9.5 us pace per 4096-wide tile pair. Engine split keeps every engine's
per-tile work under that pace so compute never trails the data:
  sync:   paced tile loads into a BUFS-slot SBUF ring (per-slot semaphores -
          HWDGE completions are NOT ordered across dma_starts)
  vector: per tile: STT (in>0.5)*tgt, accum -> loss1 col (4.43 us/4096)
  scalar: per tile: Copy(tgt) accum -> tgt col and Sign(1-2*in) accum ->
          sign col (3.7 us each + 0.28 accum-read; ~8.0 us/4096). The bin
          count is recovered on host as (count - S')/2, exact up to
          elements equal to 0.5 (~1e-8 relative).
  gpsimd: zeroes the stats tile once at start
(A tensor_reduce variant for sum(tgt) on vector measured 4.4 us/4096 -
DVE runs 1-input reduces at the same ~118 G elem/s as 2-input STTs - so
vector carried 8.8 us/tile and trailed the last DMA byte by ~9 us.)
The tile list tapers (2048, 1024, 512, 512) so the final loads' compute
tail is ~2 us. Stats are laid out as per-tile column triplets
[loss1, sign, tgt] so all but the last tile's stats DMA out overlapped
with the last tile's compute; only a [128, 3] DMA remains at the end.
Per-core stats land in a [128, 3*nt] DRAM tensor; the final tiny
reduction over cores/partitions/tiles happens on the host in float64.
"""

from contextlib import ExitStack

import numpy as np

try:
    import concourse.bass  # noqa: F401
except ImportError:  # pragma: no cover - path fallback for bare containers
    import sys

    for _p in ("/opt/trn_rl_repo", "/root/.axon_site/_ro/trn_rl_repo"):
        if _p not in sys.path:
            sys.path.insert(0, _p)

import concourse.bacc as bacc
import concourse.mybir as mybir
from concourse.bass_utils import run_bass_kernel_spmd

N_CORES = 8
FULL_SHAPE = (32, 1, 1024, 1024)
FULL_ELEMS = 32 * 1024 * 1024
PER_CORE = FULL_ELEMS // N_CORES  # 4_194_304
P = 128
FREE = PER_CORE // P  # 32768 fp32 elements per partition per tensor
THRESH = 0.5
BUFS = 4  # SBUF ring depth per tensor (4 x 16 KiB rows per partition)

# free_size per tile; sizes must sum to FREE. 4096-wide tiles (2 MiB DMAs,
# 16 KB descriptor rows) proved much more robust to cross-core HBM
# contention than 2048-wide ones. The tail tapers so the final loads'
# compute is short.
TILES = (4096, 4096, 4096, 4096, 4096, 4096, 4096, 2048, 1024, 512, 512)
assert sum(TILES) == FREE

_CACHE: dict = {}


def _build(tiles: tuple, n_cores: int):
    f32 = mybir.dt.float32
    nt = len(tiles)
    per_core = P * sum(tiles)
    max_f = max(tiles)
    nc = bacc.Bacc(
        "TRN2", target_bir_lowering=False, debug=False, num_devices=n_cores
    )
    inp = nc.dram_tensor("input", [per_core], f32, kind="ExternalInput").ap()
    tgt = nc.dram_tensor("target", [per_core], f32, kind="ExternalInput").ap()
    stats = nc.dram_tensor("stats", [P, 3 * nt], f32, kind="ExternalOutput").ap()

    ti_ring = nc.alloc_sbuf_tensor("ti_ring", [P, BUFS * max_f], f32).ap()
    tt_ring = nc.alloc_sbuf_tensor("tt_ring", [P, BUFS * max_f], f32).ap()
    # two scratch outputs per engine, alternated so consecutive same-engine
    # instructions never write the same buffer (deep-pipeline WAW)
    sd = [nc.alloc_sbuf_tensor(f"sd{i}", [P, max_f], f32).ap() for i in range(2)]
    sa = [nc.alloc_sbuf_tensor(f"sa{i}", [P, max_f], f32).ap() for i in range(2)]
    st = nc.alloc_sbuf_tensor("st", [P, 3 * nt], f32).ap()

    # per-tile DRAM offsets and cumulative consumer-instruction counts
    offs = []
    off = 0
    for tf in tiles:
        offs.append(off)
        off += P * tf
    V = [t + 1 for t in range(nt)]  # vector instrs through tile t
    S = [2 * (t + 1) for t in range(nt)]  # scalar instrs through tile t

    with ExitStack() as ctx:
        slot_sems = [
            ctx.enter_context(nc.semaphore(f"slot_sem{i}")) for i in range(BUFS)
        ]
        vec_sem = ctx.enter_context(nc.semaphore("vec_sem"))
        sc_sem = ctx.enter_context(nc.semaphore("sc_sem"))
        gp_sem = ctx.enter_context(nc.semaphore("gp_sem"))
        out_sem = ctx.enter_context(nc.semaphore("out_sem"))
        block = ctx.enter_context(nc.Block())

        @block.gpsimd
        def _(gpsimd):
            gpsimd.memset(st[:], 0.0).then_inc(gp_sem, 1)

        @block.sync
        def _(sync):
            for t, tf in enumerate(tiles):
                s_ = (t % BUFS) * max_f
                if t >= BUFS:
                    # ring slot reuse: consumers of tile t-BUFS must be done
                    sync.wait_ge(vec_sem, V[t - BUFS])
                    sync.wait_ge(sc_sem, S[t - BUFS])
                src_i = inp[offs[t] : offs[t] + P * tf].rearrange(
                    "(p f) -> p f", p=P
                )
                src_t = tgt[offs[t] : offs[t] + P * tf].rearrange(
                    "(p f) -> p f", p=P
                )
                sem = slot_sems[t % BUFS]
                sync.dma_start(out=ti_ring[:, s_ : s_ + tf], in_=src_i).then_inc(
                    sem, 16
                )
                sync.dma_start(out=tt_ring[:, s_ : s_ + tf], in_=src_t).then_inc(
                    sem, 16
                )
            # sem update on an accum instruction fires at full instruction
            # retirement (incl. the accumulator write-back), so the stats DMAs
            # can depend on the compute sems directly - no fence instructions.
            # Ship tiles [0, nt-1) overlapped with the last tile's compute;
            # only the last tile's [128, 3] triplet ships at the very end.
            sync.wait_ge(vec_sem, V[-2])
            sync.wait_ge(sc_sem, S[-2])
            sync.wait_ge(gp_sem, 1)
            head = 3 * (nt - 1)
            sync.dma_start(
                out=stats[:, :head], in_=st[:, :head]
            ).then_inc(out_sem, 16)
            sync.wait_ge(vec_sem, V[-1])
            sync.wait_ge(sc_sem, S[-1])
            sync.dma_start(
                out=stats[:, head:], in_=st[:, head:]
            ).then_inc(out_sem, 16)
            sync.wait_ge(out_sem, 32)

        @block.vector
        def _(vector):
            vector.wait_ge(gp_sem, 1)
            vi = 0
            for t, tf in enumerate(tiles):
                s_ = (t % BUFS) * max_f
                vector.wait_ge(slot_sems[t % BUFS], 32 * (t // BUFS + 1))
                if vi >= 2:
                    # scratch-reuse self-wait; satisfied by in-order retirement
                    vector.wait_ge(vec_sem, vi - 1)
                vector.scalar_tensor_tensor(
                    out=sd[t % 2][:, :tf],
                    in0=ti_ring[:, s_ : s_ + tf],
                    scalar=THRESH,
                    in1=tt_ring[:, s_ : s_ + tf],
                    op0=mybir.AluOpType.is_gt,
                    op1=mybir.AluOpType.mult,
                    accum_out=st[:, 3 * t : 3 * t + 1],
                ).then_inc(vec_sem, 1)
                vi += 1

        @block.scalar
        def _(scalar):
            scalar.wait_ge(gp_sem, 1)
            si = 0
            for t, tf in enumerate(tiles):
                s_ = (t % BUFS) * max_f
                scalar.wait_ge(slot_sems[t % BUFS], 32 * (t // BUFS + 1))
                if si >= 2:
                    scalar.wait_ge(sc_sem, si - 1)
                scalar.activation(
                    out=sa[si % 2][:, :tf],
                    in_=tt_ring[:, s_ : s_ + tf],
                    func=mybir.ActivationFunctionType.Copy,
                    accum_out=st[:, 3 * t + 2 : 3 * t + 3],
                ).then_inc(sc_sem, 1)
                si += 1
                if si >= 2:
                    scalar.wait_ge(sc_sem, si - 1)
                # Sign(1 - 2x) = -Sign(x - 0.5); bias=1.0 has a pre-registered
                # const AP; host converts the sum to a >0.5 count
                scalar.activation(
                    out=sa[si % 2][:, :tf],
                    in_=ti_ring[:, s_ : s_ + tf],
                    func=mybir.ActivationFunctionType.Sign,
                    bias=1.0,
                    scale=-2.0,
                    accum_out=st[:, 3 * t + 1 : 3 * t + 2],
                ).then_inc(sc_sem, 1)
                si += 1

    nc.compile()
    return nc


def _get_nc():
    key = (TILES, N_CORES)
    if key not in _CACHE:
        _CACHE[key] = _build(*key)
    return _CACHE[key]


def kernel(input: np.ndarray, target: np.ndarray, **run_kwargs):
    inp = np.asarray(input, dtype=np.float32).reshape(N_CORES, PER_CORE)
    tgt = np.asarray(target, dtype=np.float32).reshape(N_CORES, PER_CORE)

    nc = _get_nc()
    in_maps = [
        {"input": np.ascontiguousarray(inp[c]), "target": np.ascontiguousarray(tgt[c])}
        for c in range(N_CORES)
    ]
    res = run_bass_kernel_spmd(nc, in_maps, core_ids=list(range(N_CORES)), **run_kwargs)

    nt = len(TILES)
    inter = 0.0
    sign_sum = 0.0
    tgt_sum = 0.0
    for c in range(N_CORES):
        stats = res.results[c]["stats"].astype(np.float64).reshape(P, nt, 3)
        inter += stats[:, :, 0].sum()
        sign_sum += stats[:, :, 1].sum()
        tgt_sum += stats[:, :, 2].sum()
    # bin count from sign sums: S' = #lt - #gt -> count(>thr) = (n - S')/2
    loss2 = tgt_sum + (FULL_ELEMS - sign_sum) / 2.0

    loss1 = np.float32(2.0 * inter)
    loss2 = np.float32(loss2)
    out = (loss1, loss2)
    if run_kwargs.get("trace"):
        return out, res
    return out
